# revision 16
# baseline (speedup 1.0000x reference)
"""Trainium2 Bass kernel for nn_DoubleSubstitutionHead.

Strategy (pure data-parallel, one batch row per NeuronCore, 8 cores):

The reference computes, per batch row:
    y2 = deconv(x, W2, b2)            # [2048, 256]
    x1 = y2[sel2]                     # 1024 rows where value[:2048]==2
    y1 = deconv(x1, W1, b1)           # [8192, 256]
    x0 = y1[sel1]                     # 4096 rows where value[2048:10240]==2
    y0 = deconv(x0, W0, b0) + enc     # [32768, 256], enc = sum_c emb_c[pos_c]
    out = y0 @ lin_w + lin_b          # [32768, 17]

Key algebraic optimization: the final deconv0 (34 GFLOP) is folded through
the 17-wide output projection:
    out[u*8+m, j] = x0[u] @ V0[:, m*17+j] + sum_c Ecat_c[pos_c[u*8+m], j] + const
with V0[i, (m,j)] = sum_o W0[i,o,m] lin_w[o,j]   (256x136, built on device)
and Ecat_c = emb_c @ lin_w + (b0@lin_w + lin_b)/3  (64x17 tables).
This is a 127x FLOP reduction on the dominant term.

Dataflow is feature-major (features on SBUF partitions, tokens on the free
axis) so that the ragged compactions become free-axis gathers (GPSIMD
ap_gather).  The positional-encoding gather produces a transposed [136, u]
layout which is absorbed into the final matmul as two extra contraction
tiles against constant indicator matrices (avoiding any transpose).
"""

import os
import numpy as np
import ml_dtypes

# ---------------------------------------------------------------- constants
N, E, CS = 8, 256, 8
L2, M2 = 2048, 1024
L1, M1 = 8192, 4096
S = 43008
NV = 17            # NUM_VOCAB + 1
RES = 64
POS_BASE = S - 32768
NCORES = 8
OUT_T = 32768      # output tokens per batch row
F136 = CS * NV     # 136

_cache = {}


def _wrap16(seq):
    """Layout a 1-D list into the GPSIMD 16-partition wrap: elem i at
    [i%16, i//16]."""
    seq = np.asarray(seq)
    n = len(seq)
    assert n % 16 == 0
    return seq.reshape(n // 16, 16).T.copy()


def _rep8(w):
    """Replicate a [16, W] wrapped index block to all 8 GPSIMD core groups."""
    return np.tile(w, (8, 1)).copy()


def _ienc_consts():
    """Indicator matrices absorbed into the final matmul.

    encS row layout (partition g = m*16 + slot, bf16 pair lanes):
      slot < 8 : lanes = (Enc[j=slot], Enc[j=slot+8]) for position m
      slot == 8: lanes = (Enc[j=16], 0)
    IEnc0 maps lane-0 rows to output column (m, j): j=slot (slot<8), j=16 (slot=8)
    IEnc1 maps lane-1 rows to output column (m, j=slot+8) (slot<8)
    """
    i0 = np.zeros((128, F136), np.float32)
    i1 = np.zeros((128, F136), np.float32)
    for m in range(CS):
        for slot in range(8):
            i0[m * 16 + slot, m * NV + slot] = 1.0
            i1[m * 16 + slot, m * NV + slot + 8] = 1.0
        i0[m * 16 + 8, m * NV + 16] = 1.0
    return i0.astype(ml_dtypes.bfloat16), i1.astype(ml_dtypes.bfloat16)


# ---------------------------------------------------------------- program
def build_program(stage=5):
    import concourse.bass as bass
    import concourse.mybir as mybir
    import concourse.tile as tile
    from concourse import bacc
    from concourse.masks import make_identity

    dt = mybir.dt
    AF = mybir.ActivationFunctionType

    nc = bacc.Bacc("TRN2", target_bir_lowering=False, debug=False,
                   enable_asserts=False)

    f32, f32r, bf16, i16 = dt.float32, dt.float32r, dt.bfloat16, dt.int16

    # ---- DRAM I/O (per-core tensors; host preps layouts) ----
    x_in = nc.dram_tensor("x_in", [256, 256], f32r, kind="ExternalInput")
    w2_in = nc.dram_tensor("w2_in", [256, 2048], f32r, kind="ExternalInput")
    w1_in = nc.dram_tensor("w1_in", [256, 2048], f32r, kind="ExternalInput")
    w0_in = nc.dram_tensor("w0_in", [256, 2048], f32r, kind="ExternalInput")
    linw_in = nc.dram_tensor("linw_in", [256, NV], f32, kind="ExternalInput")
    linb_in = nc.dram_tensor("linb_in", [1, NV], f32, kind="ExternalInput")
    b2_in = nc.dram_tensor("b2_in", [128, 2], f32, kind="ExternalInput")
    b1_in = nc.dram_tensor("b1_in", [128, 2], f32, kind="ExternalInput")
    b0_in = nc.dram_tensor("b0_in", [128, 2], f32, kind="ExternalInput")
    emb_in = nc.dram_tensor("emb_in", [3, 64, 256], f32r, kind="ExternalInput")
    idx2_in = nc.dram_tensor("idx2_in", [128, 2 * M2 // 16], i16, kind="ExternalInput")
    idx1_in = nc.dram_tensor("idx1_in", [128, M1 // 16], i16, kind="ExternalInput")
    posw_in = nc.dram_tensor("posw_in", [3, 128, M1 // 16], i16, kind="ExternalInput")
    ienc_in = nc.dram_tensor("ienc_in", [2, 128, F136], bf16, kind="ExternalInput")

    out_d = nc.dram_tensor("out", [OUT_T, NV], f32, kind="ExternalOutput")
    dbg_d = None
    if stage < 5:
        dbg_d = nc.dram_tensor("dbg", [128, 9544], f32, kind="ExternalOutput")

    class _StageDone(Exception):
        pass

    import contextlib
    with tile.TileContext(nc) as tc:
      with contextlib.suppress(_StageDone):
        with (
            tc.tile_pool(name="persist", bufs=1) as pp,
            tc.tile_pool(name="wx", bufs=1) as wx,       # w0sb then x0pk (reuse)
            tc.tile_pool(name="ye", bufs=1) as ye,       # y2sb then enc_a (reuse)
            tc.tile_pool(name="small", bufs=1) as sp,
            tc.tile_pool(name="ob", bufs=4) as ob,
            tc.tile_pool(name="psA", bufs=3, space="PSUM") as psA,
            tc.tile_pool(name="psT", bufs=2, space="PSUM") as psT,
            tc.tile_pool(name="psS", bufs=3, space="PSUM") as psS,
        ):
            # ---------- constant + weight loads ----------
            ident_f = sp.tile([128, 128], f32)
            make_identity(nc, ident_f[:])
            ident = sp.tile([128, 128], f32r)
            nc.vector.tensor_copy(ident[:], ident_f[:])

            xsb = sp.tile([128, 2, 256], f32r)     # [t-part, t-half, i]
            nc.sync.dma_start(xsb[:], x_in.ap().rearrange("(a p) i -> p a i", p=128))

            w2sb = pp.tile([128, 2, 2048], f32r)   # [i-part, i-half, (o,k)]
            nc.sync.dma_start(w2sb[:], w2_in.ap().rearrange("(h p) f -> p h f", p=128))
            w1sb = pp.tile([128, 2, 2048], f32r)
            nc.sync.dma_start(w1sb[:], w1_in.ap().rearrange("(h p) f -> p h f", p=128))
            w0sb = wx.tile([128, 2, 2048], f32r, tag="wx")
            nc.sync.dma_start(w0sb[:], w0_in.ap().rearrange("(h p) f -> p h f", p=128))

            linwsb = sp.tile([128, 2, NV], f32)
            nc.sync.dma_start(linwsb[:], linw_in.ap().rearrange("(h p) j -> p h j", p=128))
            linbsb = sp.tile([1, NV], f32)
            nc.sync.dma_start(linbsb[:], linb_in.ap())
            b2sb = sp.tile([128, 2], f32)
            nc.sync.dma_start(b2sb[:], b2_in.ap())
            b1sb = sp.tile([128, 2], f32)
            nc.sync.dma_start(b1sb[:], b1_in.ap())
            b0sb = sp.tile([128, 2], f32)
            nc.sync.dma_start(b0sb[:], b0_in.ap())
            embsb = sp.tile([64, 3, 256], f32r)
            for c in range(3):
                nc.sync.dma_start(embsb[:, c], emb_in.ap()[c])

            idx2 = sp.tile([128, 2 * M2 // 16], i16)
            nc.sync.dma_start(idx2[:], idx2_in.ap())
            idx1 = sp.tile([128, M1 // 16], i16)
            nc.sync.dma_start(idx1[:], idx1_in.ap())
            posw = sp.tile([128, 3, M1 // 16], i16)
            for c in range(3):
                nc.sync.dma_start(posw[:, c], posw_in.ap()[c])
            ienc = sp.tile([128, 2, F136], bf16)
            for h in range(2):
                nc.sync.dma_start(ienc[:, h], ienc_in.ap()[h])

            ones1 = sp.tile([1, 1], f32)
            nc.vector.memset(ones1[:], 1.0)

            # ---------- x transpose: xt[p, h, t] = x[t, h*128+p] ----------
            xt = sp.tile([128, 2, 256], f32r)
            for a in range(2):          # token half
                for h in range(2):      # feature half
                    ps = psT.tile([128, 128], f32, tag="tp")
                    nc.tensor.transpose(
                        ps[:].bitcast(f32r),
                        xsb[:, a, h * 128:(h + 1) * 128],
                        ident[:])
                    if (a + h) % 2:
                        nc.scalar.copy(xt[:, h, a * 128:(a + 1) * 128], ps[:])
                    else:
                        nc.vector.tensor_copy(xt[:, h, a * 128:(a + 1) * 128], ps[:])

            # ---------- W0 transpose: w0t[p, g, k, i] = W0[i, g*128+p, k] ----
            w0t = pp.tile([128, 2, 8, 256], f32)
            w0v = w0sb[:].rearrange("p h (o k) -> p h o k", k=8)
            for g in range(2):
                for k in range(8):
                    for h in range(2):
                        ps = psT.tile([128, 128], f32, tag="tp")
                        nc.tensor.transpose(
                            ps[:].bitcast(f32r),
                            w0v[:, h, g * 128:(g + 1) * 128, k],
                            ident[:])
                        if (g + k + h) % 2:
                            nc.scalar.copy(w0t[:, g, k, h * 128:(h + 1) * 128], ps[:])
                        else:
                            nc.vector.tensor_copy(w0t[:, g, k, h * 128:(h + 1) * 128], ps[:])

            # ---------- V0 fold: v0r[p, iT, m*17+j] (bf16) ----------
            v0r = sp.tile([128, 2, F136], bf16)
            for k in range(8):
                for iT in range(2):
                    ps = psS.tile([128, NV], f32, tag="s")
                    for g in range(2):
                        nc.tensor.matmul(
                            ps[:], w0t[:, g, k, iT * 128:(iT + 1) * 128],
                            linwsb[:, g], start=(g == 0), stop=(g == 1))
                    if (k + iT) % 2:
                        nc.scalar.copy(v0r[:, iT, k * NV:(k + 1) * NV], ps[:])
                    else:
                        nc.vector.tensor_copy(v0r[:, iT, k * NV:(k + 1) * NV], ps[:])

            # ---------- bconst = b0 @ lin_w + lin_b, transposed to [17, 1] ---
            psb = psS.tile([1, NV], f32, tag="s")
            for h in range(2):
                nc.tensor.matmul(psb[:], b0sb[:, h:h + 1], linwsb[:, h],
                                 start=(h == 0), stop=(h == 1))
            bconst = sp.tile([1, NV], f32)
            nc.vector.tensor_add(bconst[:], psb[:], linbsb[:])
            psbT = psS.tile([NV, 1], f32, tag="s")
            nc.tensor.matmul(psbT[:], bconst[:], ones1[:])
            bconstT = sp.tile([NV, 1], f32)
            nc.vector.tensor_scalar_mul(bconstT[:], psbT[:], 1.0 / 3.0)

            # ---------- Ecat tables ----------
            # embT_c[p, h, v] = emb[c, v, h*128+p]
            embT = sp.tile([128, 3, 2, 64], f32)
            for c in range(3):
                for h in range(2):
                    ps = psT.tile([128, 64], f32, tag="tp")
                    nc.tensor.transpose(
                        ps[:].bitcast(f32r),
                        embsb[:, c, h * 128:(h + 1) * 128],
                        ident[:64, :64])
                    if (c + h) % 2:
                        nc.scalar.copy(embT[:, c, h], ps[:])
                    else:
                        nc.vector.tensor_copy(embT[:, c, h], ps[:])

            # EcatT_c[j, v] = (emb_c @ lin_w)^T + bconst/3 ; packed bf16 pairs
            table = sp.tile([128, 3, 64], f32)  # [(m,slot), c, v] as packed pairs
            nc.vector.memset(table[:], 0.0)
            packed = sp.tile([16, 3, 128], bf16)  # [slot, c, (v,lane)]
            nc.vector.memset(packed[:], 0.0)
            for c in range(3):
                psE = psS.tile([NV, 64], f32, tag="s")
                for h in range(2):
                    nc.tensor.matmul(psE[:], linwsb[:, h], embT[:, c, h],
                                     start=(h == 0), stop=(h == 1))
                ecatT = sp.tile([NV, 64], f32)
                nc.vector.tensor_scalar_add(ecatT[:], psE[:], bconstT[:])
                ecatB = sp.tile([NV, 64], bf16)
                nc.vector.tensor_copy(ecatB[:], ecatT[:])
                pv = packed[:].rearrange("s c (v l) -> s c v l", l=2)
                # DMA (not DVE): partition starts 8/16 are illegal on DVE
                nc.sync.dma_start(pv[0:8, c, :, 0], ecatB[0:8])    # j = slot
                nc.sync.dma_start(pv[0:8, c, :, 1], ecatB[8:16])   # j = slot+8
                nc.sync.dma_start(pv[8:9, c, :, 0], ecatB[16:17])  # j = 16
                # replicate to the 8 m-groups
                for m in range(8):
                    nc.sync.dma_start(
                        table[m * 16:m * 16 + 9, c].bitcast(bf16), packed[0:9, c])

            if stage < 5:
                nc.sync.dma_start(dbg_d.ap()[:, 0:192], table[:])
                nc.sync.dma_start(dbg_d.ap()[:, 192:328],
                                  v0r[:].bitcast(f32).rearrange("p a b -> p (a b)"))
            if stage < 2:
                raise _StageDone()

            # ---------- enc gather c=0 (early: only needs table+posw) ----
            enc_a = pp.tile([128, 4096], f32)
            enc_b = pp.tile([128, 4096], f32)
            nc.gpsimd.ap_gather(enc_a[:], table[:, 0], posw[:, 0],
                                channels=128, num_elems=64, d=1, num_idxs=M1)

            # ---------- deconv2 ----------
            # y2sb[p, oh, k*256 + t] = y2[feat oh*128+p, token t*8+k]
            y2sb = ye.tile([128, 2, 2048], f32r, tag="ye")
            w2v = w2sb[:].rearrange("p h (o k) -> p h o k", k=8)
            for k in range(8):
                for oh in range(2):
                    ps = psA.tile([128, 256], f32, tag="mm")
                    for h in range(2):
                        nc.tensor.matmul(
                            ps[:],
                            w2v[:, h, oh * 128:(oh + 1) * 128, k],
                            xt[:, h],
                            start=(h == 0), stop=(h == 1))
                    if (k + oh) % 2:
                        nc.scalar.add(y2sb[:, oh, k * 256:(k + 1) * 256], ps[:],
                                      b2sb[:, oh:oh + 1])
                    else:
                        nc.vector.tensor_scalar_add(
                            y2sb[:, oh, k * 256:(k + 1) * 256], ps[:],
                            b2sb[:, oh:oh + 1])

            # ---------- x1 gather ----------
            # ap_gather ucode crashes on float32r dtype -> gather into an f32
            # tile, then copy into the f32r tile the matmuls consume
            # (walrus requires fp32r matmul inputs to be produced as f32r).
            x1f = sp.tile([128, 2, 1024], f32)
            nc.gpsimd.ap_gather(
                x1f[:].rearrange("p a b -> p (a b)"),
                y2sb[:].bitcast(f32).rearrange("p a b -> p (a b)"), idx2[:],
                channels=128, num_elems=4096, d=1, num_idxs=2 * M2)
            x1sb = sp.tile([128, 2, 1024], f32r)
            nc.vector.tensor_copy(x1sb[:, 0], x1f[:, 0])
            nc.scalar.copy(x1sb[:, 1], x1f[:, 1])

            # ---------- enc gathers c=1,2 + merge (overlap deconv1 on Pool) --
            nc.gpsimd.ap_gather(enc_b[:], table[:, 1], posw[:, 1],
                                channels=128, num_elems=64, d=1, num_idxs=M1)
            nc.vector.tensor_add(enc_a[:].bitcast(bf16), enc_a[:].bitcast(bf16),
                                 enc_b[:].bitcast(bf16))
            nc.gpsimd.ap_gather(enc_b[:], table[:, 2], posw[:, 2],
                                channels=128, num_elems=64, d=1, num_idxs=M1)
            nc.vector.tensor_add(enc_a[:].bitcast(bf16), enc_a[:].bitcast(bf16),
                                 enc_b[:].bitcast(bf16))

            if stage < 5:
                nc.sync.dma_start(
                    dbg_d.ap()[:, 328:2376],
                    x1sb[:].bitcast(f32).rearrange("p a b -> p (a b)"))
            if stage < 3:
                raise _StageDone()

            # ---------- deconv1 (outputs packed bf16 pairs) ----------
            # y1pk word [p, k*1024 + t] lanes = (y1[p, .], y1[p+128, .])
            y1pk = pp.tile([128, 16384], bf16)
            y1v = y1pk[:].rearrange("p (w l) -> p w l", l=2)
            w1v = w1sb[:].rearrange("p h (o k) -> p h o k", k=8)
            for k in range(8):
                for oh in range(2):
                    for nt in range(2):
                        ps = psA.tile([128, 512], f32, tag="mm")
                        for h in range(2):
                            nc.tensor.matmul(
                                ps[:],
                                w1v[:, h, oh * 128:(oh + 1) * 128, k],
                                x1sb[:, h, nt * 512:(nt + 1) * 512],
                                start=(h == 0), stop=(h == 1))
                        dst = y1v[:, k * 1024 + nt * 512:k * 1024 + (nt + 1) * 512, oh]
                        if (k + oh + nt) % 2:
                            nc.scalar.add(dst, ps[:], b1sb[:, oh:oh + 1])
                        else:
                            nc.vector.tensor_scalar_add(dst, ps[:], b1sb[:, oh:oh + 1])

            if stage < 5:
                nc.sync.dma_start(dbg_d.ap()[:, 2376:2376 + 4096],
                                  y1pk[:, 0:8192].bitcast(f32))
            if stage < 4:
                raise _StageDone()

            # ---------- x0 gather (split in 4) pipelined with stage0 --------
            x0pk = wx.tile([128, 4096], f32, tag="wx")
            x0v = x0pk[:].bitcast(bf16).rearrange("p (u l) -> p u l", l=2)
            encv = enc_a[:].bitcast(bf16).rearrange("p (u l) -> p u l", l=2)
            # out rows ((cg*4 + c4)*128 + u)*8 + m, col j ->
            #   grouped view [cg, u, (c4 m j)]
            outg = out_d.ap().rearrange("(cg c4 u m) j -> cg u c4 (m j)",
                                        c4=4, u=128, m=CS)
            nc.gpsimd.ap_gather(
                x0pk[:], y1pk[:].bitcast(f32), idx1[:],
                channels=128, num_elems=8192, d=1, num_idxs=M1)
            if stage >= 5:
                for cg in range(8):
                    osb = ob.tile([128, 4 * F136], f32, tag="ot")
                    for c4 in range(4):
                        ch = cg * 4 + c4
                        ps = psS.tile([128, F136], f32, tag="s")
                        us = slice(ch * 128, (ch + 1) * 128)
                        nc.tensor.matmul(ps[:], x0v[:, us, 0], v0r[:, 0],
                                         start=True, stop=False)
                        nc.tensor.matmul(ps[:], x0v[:, us, 1], v0r[:, 1],
                                         start=False, stop=False)
                        nc.tensor.matmul(ps[:], encv[:, us, 0], ienc[:, 0],
                                         start=False, stop=False)
                        nc.tensor.matmul(ps[:], encv[:, us, 1], ienc[:, 1],
                                         start=False, stop=True)
                        dst = osb[:, c4 * F136:(c4 + 1) * F136]
                        if ch % 2:
                            nc.scalar.copy(dst, ps[:])
                        else:
                            nc.vector.tensor_copy(dst, ps[:])
                    nc.sync.dma_start(
                        outg[cg], osb[:].rearrange("p (c4 f) -> p c4 f", c4=4))

            if stage < 5:
                nc.sync.dma_start(dbg_d.ap()[:, 2376 + 4096:2376 + 6144],
                                  x0pk[:, 0:2048])
                nc.sync.dma_start(dbg_d.ap()[:, 2376 + 6144:2376 + 6144 + 1024],
                                  enc_a[:, 0:1024])
                raise _StageDone()

    nc.compile()
    return nc


# ---------------------------------------------------------------- host prep
def make_in_map(inputs, n):
    """Build the per-core input map for batch row n (host-side relayout)."""
    x = np.ascontiguousarray(inputs["x"][n], np.float32)
    value = inputs["value"][n]
    pos = inputs["pos"][n]

    sel2 = np.nonzero(value[:L2] == 2)[0][:M2]
    s2 = (sel2 % CS) * 256 + sel2 // CS
    src2 = np.concatenate([s2, 2048 + s2]).astype(np.int16)
    sel1 = np.nonzero(value[L2:L2 + L1] == 2)[0][:M1]
    src1 = ((sel1 % CS) * 1024 + sel1 // CS).astype(np.int16)

    u = np.arange(M1)
    posw = np.empty((3, 128, M1 // 16), np.int16)
    for c in range(3):
        pc = pos[POS_BASE:, c]
        for m in range(CS):
            posw[c, m * 16:(m + 1) * 16] = _wrap16(pc[u * CS + m])

    i0, i1 = _ienc_consts()
    return {
        "x_in": x,
        "w2_in": np.ascontiguousarray(inputs["W2"].reshape(E, E * CS), np.float32),
        "w1_in": np.ascontiguousarray(inputs["W1"].reshape(E, E * CS), np.float32),
        "w0_in": np.ascontiguousarray(inputs["W0"].reshape(E, E * CS), np.float32),
        "linw_in": np.ascontiguousarray(inputs["lin_w"], np.float32),
        "linb_in": np.ascontiguousarray(inputs["lin_b"].reshape(1, NV), np.float32),
        "b2_in": np.ascontiguousarray(inputs["b2"].reshape(2, 128).T, np.float32),
        "b1_in": np.ascontiguousarray(inputs["b1"].reshape(2, 128).T, np.float32),
        "b0_in": np.ascontiguousarray(inputs["b0"].reshape(2, 128).T, np.float32),
        "emb_in": np.ascontiguousarray(inputs["emb"], np.float32),
        "idx2_in": np.ascontiguousarray(_rep8(_wrap16(src2))),
        "idx1_in": np.ascontiguousarray(_rep8(_wrap16(src1))),
        "posw_in": posw,
        "ienc_in": np.stack([i0, i1]),
    }


# ---------------------------------------------------------------- entry
def kernel(**inputs):
    from concourse import bass_utils

    if "nc" not in _cache:
        _cache["nc"] = build_program()
    nc = _cache["nc"]

    in_maps = [make_in_map(inputs, n) for n in range(NCORES)]
    res = bass_utils.run_bass_kernel_spmd(nc, in_maps, core_ids=list(range(NCORES)))
    out = np.stack([res.results[n]["out"] for n in range(NCORES)])
    return out.astype(np.float32)
